# revision 35
# baseline (speedup 1.0000x reference)
"""HQQ+SVD linear kernel for Trainium2, 8-way token-parallel (data parallel).

y[b,s,o] = sum_i x[b,s,i] * W_f[o,i] + bias[o]
W_f = (W_q - zp)*scale  (per-group dequant)  + svd_up @ svd_down

Sharding: tokens (B*S = 8192) split across 8 cores (1024 each); every core
computes all 4096 out-features for its token shard. This minimizes per-core
HBM traffic: each core reads its x shard (bf16, 8.4 MB) + the full quantized
weight (int8, 16.8 MB) and writes its y shard (bf16, 8.4 MB) — ~34 MB/core
vs ~160 MB/core for the column-parallel alternative (where all of x is read
by every core).

Host-side prep (layout/dtype transforms only, done once per call): x
transposed + cast to bf16, W_q narrowed to int8 [OUT, IN], sz =
[scale | -zp*scale] concatenated, SVD factors transposed and cast to bf16,
bias broadcast to [128, OUT] bf16. Output comes back bf16 and is upcast on
host.

Per-core device program (all matmul operands bf16, psum fp32), ~0.5 ms,
PE-bound at ~95% occupancy (437 us of it is the unavoidable bf16 GEMM):
  1. x arrives pre-transposed: one clean 2KB-descriptor DMA on the SP
     queue into resident xT [128k, 32kt, 1024t] bf16 (64 KB/partition).
     Small tensors ride the otherwise-idle gpsimd DMA queue.
  2. t1T[r,t] = svd_downT^T @ xT (rank 32) once on PE, overlapped with the
     chunk-0 weight prep via three psum groups held open (the SVD close is
     deferred until t1T exists).
  3. For each o-chunk of 512 (software-pipelined ahead of the matmuls):
     DMA W_q int8 o-tiles, DVE dequant (fused mult+add per 128-wide group,
     per-partition scale/negzs scalars) -> bf16 wf, then transpose to
     wT [128k, 32kt, 512o]: chunk 0 on PE (idle during the prologue
     anyway), chunks 1+ via the XBAR DMA-transpose (14ns per 16x128 tile)
     so steady-state PE does nothing but matmuls.
  4. Main matmuls: psum[128t, 512o] accumulates 32 k-tiles plus the rank-32
     SVD correction (t1T as lhsT, svd_upT as rhs); DVE drains psum + bias
     -> bf16 y slab; DMA out.

The "null" variant has the identical I/O signature but a trivial body; the
test harness uses it to difference away per-execution dispatch overhead.
"""

import os
import sys

sys.path.insert(0, "/opt/trn_rl_repo")

import numpy as np
import ml_dtypes

import concourse.bass as bass
import concourse.mybir as mybir
from concourse import bacc
from concourse.masks import make_identity
from concourse.tile import TileContext
from concourse.bass_utils import run_bass_kernel_spmd

OUT, IN, RANK, NG, GS = 4096, 4096, 32, 32, 128
B, S = 4, 2048
T = B * S  # 8192 tokens
N_CORES = 8
TSH = T // N_CORES  # 1024 tokens per core

P = 128
N_KT = IN // P  # 32 k-tiles
N_TS = TSH // P  # 8 token slabs per core
OC = 512  # out-features chunk streamed through SBUF
N_OC = OUT // OC  # 8 chunks
N_OT = OC // P  # 4 o-tiles per chunk
F32 = mybir.dt.float32
BF16 = mybir.dt.bfloat16
I8 = mybir.dt.int8

BF16_NP = ml_dtypes.bfloat16


def build(nc: bass.Bass, variant: str = "dp"):
    x = nc.dram_tensor("x", [IN, TSH], BF16, kind="ExternalInput")
    wq = nc.dram_tensor("wq", [OUT, IN], I8, kind="ExternalInput")
    sz = nc.dram_tensor("sz", [OUT, 2 * NG], F32, kind="ExternalInput")
    svd_dT = nc.dram_tensor("svd_dT", [IN, RANK], BF16, kind="ExternalInput")
    svd_uT = nc.dram_tensor("svd_uT", [RANK, OUT], BF16, kind="ExternalInput")
    bias_bc = nc.dram_tensor("bias_bc", [P, OUT], BF16, kind="ExternalInput")
    y = nc.dram_tensor("y", [TSH, OUT], BF16, kind="ExternalOutput")

    if variant == "null":
        # same I/O signature, trivial body: touch each input, write all of y
        with TileContext(nc) as tc:
            with tc.tile_pool(name="nullp", bufs=2) as pool:
                t = pool.tile([P, OUT], BF16)
                nc.sync.dma_start(t[:, :TSH], x.ap()[:P, :TSH])
                for name, ap, shp, dt in (
                    ("wq", wq, (P, OUT), I8),
                    ("sz", sz, (P, 2 * NG), F32),
                    ("sd", svd_dT, (P, RANK), BF16),
                    ("su", svd_uT, (RANK, OUT), BF16),
                    ("b", bias_bc, (P, OUT), BF16),
                ):
                    tt_ = pool.tile(list(shp), dt, tag=f"n_{name}")
                    nc.sync.dma_start(tt_[:], ap.ap()[: shp[0], : shp[1]])
                for ts in range(N_TS):
                    nc.sync.dma_start(y.ap()[ts * P : (ts + 1) * P, :], t[:])
        return nc

    with TileContext(nc) as tc:
        with (
            tc.tile_pool(name="consts", bufs=1) as consts,
            tc.tile_pool(name="xTp", bufs=1) as p_xT,
            tc.tile_pool(name="t1p", bufs=1) as p_t1,
        ):
            # xT resident: [128 k-part, 32 kt, 1024 t] bf16. One clean
            # 2KB-descriptor DMA, issued first on the SP queue: every main
            # matmul reads xT, so this transfer is the prologue gate.
            xT = p_xT.tile([P, N_KT, TSH], BF16)
            nc.sync.dma_start(xT[:], x.ap().rearrange("(a p) t -> p a t", p=P))

            # small tensors go on the (otherwise idle) gpsimd DMA queue so
            # they don't serialize behind xT: dequant needs scale/negzs/wq
            # early, and DMAs on one engine queue run strictly in order.
            # combined scale|negzs: [128 o-part, 32 o-tile, 64] f32 — one
            # DMA halves the serial prologue transfer chain.
            sz_sb = consts.tile([P, OUT // P, 2 * NG], F32)
            nc.gpsimd.dma_start(sz_sb[:], sz.ap().rearrange("(a p) g -> p a g", p=P))
            scale_sb = sz_sb  # scalar g -> [:, go, g]; negzs g -> [:, go, NG+g]

            # identity for the chunk-0 PE transposes: emitted after the DMA
            # issues so the gpsimd ALU work doesn't block the DMA queue.
            identity = consts.tile([P, P], BF16)
            make_identity(nc, identity)

            # svd_downT: [128 k-part, 32 kt, 32 r] bf16 (DMA issued after
            # chunk 0's wq tiles — not needed until t1 at ~25us)
            svd_dT_sb = consts.tile([P, N_KT, RANK], BF16)
            svd_uT_sb = consts.tile([RANK, OUT], BF16)
            bias_sb = consts.tile([P, OUT], BF16)
            # t1T resident: [32 r, 1024 t] bf16
            t1T = p_t1.tile([RANK, TSH], BF16)

            # ---- main: W-prep pipelined ahead of matmuls ----
            with (
                tc.tile_pool(name="wq_sb", bufs=4) as p_wq,
                tc.tile_pool(name="wf_sb", bufs=3) as p_wf,
                tc.tile_pool(name="wT_sb", bufs=2) as p_wT,
                tc.tile_pool(name="ysb", bufs=3) as p_y,
                tc.tile_pool(name="ps_wt", bufs=2, space="PSUM") as p_pswt,
                tc.tile_pool(name="ps_t1", bufs=2, space="PSUM") as p_pst1,
                tc.tile_pool(name="ps_y", bufs=4, space="PSUM") as p_psy,
            ):
                wT_tiles = {}

                def prep_chunk(oc, pe_transpose=False):
                    wT = p_wT.tile([P, N_KT, OC], BF16, tag="wT")
                    wT_tiles[oc] = wT
                    wfs = []
                    for ot in range(N_OT):
                        go = oc * N_OT + ot  # global o-tile
                        wq_t = p_wq.tile([P, IN], I8, tag="wq")
                        nc.gpsimd.dma_start(wq_t[:], wq.ap()[go * P : (go + 1) * P, :])
                        wf_t = p_wf.tile([P, IN], BF16, tag="wf")
                        wfs.append(wf_t)
                        # fused per-group dequant on DVE
                        for g in range(NG):
                            nc.vector.tensor_scalar(
                                out=wf_t[:, g * GS : (g + 1) * GS],
                                in0=wq_t[:, g * GS : (g + 1) * GS],
                                scalar1=sz_sb[:, go, g : g + 1],
                                scalar2=sz_sb[:, go, NG + g : NG + g + 1],
                                op0=mybir.AluOpType.mult,
                                op1=mybir.AluOpType.add,
                            )
                        if not pe_transpose:
                            # XBAR DMA transpose: wf [128o, 4096k] ->
                            # wT[k, kt, o] (16x128 tiles, ~3.6us of
                            # DMA-engine time per o-tile), keeping PE free
                            # for matmuls and ACT free of copies.
                            nc.scalar.dma_start_transpose(
                                wT[:, :, ot * P : (ot + 1) * P], wf_t[:]
                            )
                        else:
                            # chunk 0 only: PE is otherwise idle during the
                            # prologue (waiting on the xT DMA), and the XBAR
                            # path would queue behind xT on the DMA engines —
                            # so transpose on PE + copy on ACT.
                            for kg in range(N_KT // 4):
                                ps_t = p_pswt.tile([P, 512], BF16, tag="wtp")
                                for j in range(4):
                                    kt = kg * 4 + j
                                    nc.tensor.transpose(
                                        ps_t[:, j * P : (j + 1) * P],
                                        wf_t[:, kt * P : (kt + 1) * P],
                                        identity[:],
                                    )
                                nc.scalar.copy(
                                    wT[:, kg * 4 : kg * 4 + 4, ot * P : (ot + 1) * P],
                                    ps_t[:].rearrange("p (a o) -> p a o", a=4),
                                )


                def mm_group(ps_y, oc, ts, wT, with_svd):
                    for kt in range(N_KT):
                        nc.tensor.matmul(
                            ps_y[:],
                            xT[:, kt, ts * P : (ts + 1) * P],
                            wT[:, kt, :],
                            start=(kt == 0),
                            stop=False,
                        )
                    if with_svd:
                        svd_close(ps_y, oc, ts)

                def svd_close(ps_y, oc, ts):
                    # rank-32 SVD correction folded into the same psum
                    nc.tensor.matmul(
                        ps_y[:],
                        t1T[:, ts * P : (ts + 1) * P],
                        svd_uT_sb[:, oc * OC : (oc + 1) * OC],
                        start=False,
                        stop=True,
                    )

                def drain(ps_y, oc, ts):
                    y_sb = p_y.tile([P, OC], BF16, tag="ysb")
                    nc.vector.tensor_tensor(
                        out=y_sb[:],
                        in0=ps_y[:],
                        in1=bias_sb[:, oc * OC : (oc + 1) * OC],
                        op=mybir.AluOpType.add,
                    )
                    nc.sync.dma_start(
                        y.ap()[ts * P : (ts + 1) * P, oc * OC : (oc + 1) * OC],
                        y_sb[:],
                    )

                # Chunk 0 is special-cased to keep PE busy while the xT DMA
                # (needed by t1) completes: open the first three psum groups
                # without their SVD close, emit chunk-1 prep, then t1, then
                # close + drain them.
                prep_chunk(0, pe_transpose=True)
                # svd_dT/svd_uT/bias are not needed until t1 / the first
                # svd-close/drain; issue them behind chunk 0's wq tiles.
                nc.gpsimd.dma_start(
                    svd_dT_sb[:], svd_dT.ap().rearrange("(a p) r -> p a r", p=P)
                )
                nc.gpsimd.dma_start(svd_uT_sb[:], svd_uT.ap())
                nc.gpsimd.dma_start(bias_sb[:], bias_bc.ap())
                prep_chunk(1)
                wT0 = wT_tiles.pop(0)
                N_OPEN = 3  # == ps_y bufs
                open_ps = []
                for ts in range(N_OPEN):
                    ps_y = p_psy.tile([P, OC], F32, tag="y")
                    mm_group(ps_y, 0, ts, wT0, with_svd=False)
                    open_ps.append(ps_y)

                # t1T[r, t] = sum_k svd_dT[k, r] * xT[k, t]
                for tc_i in range(TSH // 512):
                    ps1 = p_pst1.tile([RANK, 512], F32, tag="t1")
                    for kt in range(N_KT):
                        nc.tensor.matmul(
                            ps1[:],
                            svd_dT_sb[:, kt, :],
                            xT[:, kt, tc_i * 512 : (tc_i + 1) * 512],
                            start=(kt == 0),
                            stop=(kt == N_KT - 1),
                        )
                    nc.scalar.copy(t1T[:, tc_i * 512 : (tc_i + 1) * 512], ps1[:])

                for ts in range(N_OPEN):
                    svd_close(open_ps[ts], 0, ts)
                    drain(open_ps[ts], 0, ts)
                for ts in range(N_OPEN, N_TS):
                    ps_y = p_psy.tile([P, OC], F32, tag="y")
                    mm_group(ps_y, 0, ts, wT0, with_svd=True)
                    drain(ps_y, 0, ts)
                prep_chunk(2)

                for oc in range(1, N_OC):
                    wT = wT_tiles.pop(oc)
                    for ts in range(N_TS):
                        ps_y = p_psy.tile([P, OC], F32, tag="y")
                        mm_group(ps_y, oc, ts, wT, with_svd=True)
                        drain(ps_y, oc, ts)
                    if oc + 2 < N_OC:
                        prep_chunk(oc + 2)
    return nc


_NC_CACHE = {}


def _get_nc(variant: str = "dp"):
    if variant not in _NC_CACHE:
        nc = bacc.Bacc(None, target_bir_lowering=False)
        build(nc, variant)
        nc.compile()
        _NC_CACHE[variant] = nc
    return _NC_CACHE[variant]


def _in_maps(x, W_q, svd_up, svd_down, scale, zero_point, bias, variant="dp"):
    x2 = np.asarray(x, dtype=np.float32).reshape(T, IN).astype(BF16_NP)
    # per-core pre-transposed x shard: [IN, TSH] bf16
    xT = [
        np.ascontiguousarray(x2[c * TSH : (c + 1) * TSH].T) for c in range(N_CORES)
    ]
    wq8 = np.asarray(W_q, dtype=np.int32).reshape(OUT, IN).astype(np.int8)
    scale_f = np.asarray(scale, dtype=np.float32)
    sz = np.ascontiguousarray(
        np.concatenate(
            [scale_f, -(np.asarray(zero_point, dtype=np.float32) * scale_f)], axis=1
        )
    )
    svd_dT = np.ascontiguousarray(np.asarray(svd_down, dtype=np.float32).T).astype(
        BF16_NP
    )
    svd_uT = np.ascontiguousarray(np.asarray(svd_up, dtype=np.float32).T).astype(
        BF16_NP
    )
    bias_bc = np.ascontiguousarray(
        np.broadcast_to(
            np.asarray(bias, dtype=np.float32).astype(BF16_NP)[None, :], (P, OUT)
        )
    )
    maps = []
    for c in range(N_CORES):
        maps.append(
            {
                "x": xT[c],
                "wq": wq8,
                "sz": sz,
                "svd_dT": svd_dT,
                "svd_uT": svd_uT,
                "bias_bc": bias_bc,
            }
        )
    return maps


def _run(in_maps, variant="dp", **kw):
    nc = _get_nc(variant)
    return run_bass_kernel_spmd(nc, in_maps, core_ids=list(range(N_CORES)), **kw)


VARIANT = os.environ.get("KERNEL_VARIANT", "dp")


def kernel(x, W_q, svd_up, svd_down, scale, zero_point, bias):
    res = _run(
        _in_maps(x, W_q, svd_up, svd_down, scale, zero_point, bias, VARIANT),
        variant=VARIANT,
    )
    y = np.concatenate([res.results[c]["y"] for c in range(N_CORES)], axis=0)
    return y.astype(np.float32).reshape(B, S, OUT)
